# revision 1
# baseline (speedup 1.0000x reference)
"""Multi-head causal attention (B=4, T=2048, D=1024, H=16, hd=64) on 8 trn2 cores.

Sharding: core = (batch, head_group): 4 batches x 2 head-groups of 8 heads.
Each core computes its batch's attention for its 8 heads plus the partial
output projection; the host sums the two head-group partials per batch and
adds the output bias.

Per-core kernel (all activations kept transposed, [feature, token]):
  QT = Wq_s^T @ x^T        [512, 2048]   (PE, fp32r)
  KT = Wk_s^T @ x^T        [512, 2048]
  V  = x @ Wv_s            [2048, 512]   (token-partition layout, +ones col)
  per head h, i-chunk c (1024 wide), j-tile t (128 rows):
    S^T[j, i] = KT_h[:, jtile]^T-contract QT_h    (PE, causal extents only)
    expS = exp(S^T / 8)                           (ACT, PSUM->SBUF)
    causal mask on the 128-col diagonal block     (GPSIMD affine_select)
    ctxT_aug += V_aug[jtile]^T-contract expS      (PE, PSUM accumulate;
                                                   row 64 = softmax denom)
    ctx = ctxT[0:64] * (1/denom)                  (DVE + GPSIMD broadcast)
  out_partial = ctx^T-contract Wo_s               [2048, 1024]
"""

import os
import sys

sys.path.insert(0, "/opt/trn_rl_repo")

import numpy as np

B = 4
T = 2048
D = 1024
H = 16
HD = 64
NCORES = 8
HPC = 8          # heads per core
DPC = HPC * HD   # 512
KT = D // 128    # 8 k-tiles
NT = T // 128    # 16 token tiles

_CACHE = {}
LAST_RESULTS = None


def _build_program():
    from contextlib import ExitStack

    import concourse.bass as bass
    import concourse.tile as tile
    from concourse import bacc, mybir

    f32 = mybir.dt.float32
    f32r = mybir.dt.float32r
    bf16 = mybir.dt.bfloat16
    Exp = mybir.ActivationFunctionType.Exp

    def r(ap):
        return ap

    nc = bacc.Bacc(
        "TRN2", target_bir_lowering=False, debug=False, num_devices=NCORES
    )
    xT = nc.dram_tensor("xT", [D, T], f32r, kind="ExternalInput").ap()
    wq_d = nc.dram_tensor("wq", [D, DPC], f32r, kind="ExternalInput").ap()
    wk_d = nc.dram_tensor("wk", [D, DPC], f32r, kind="ExternalInput").ap()
    wv_d = nc.dram_tensor("wv", [D, DPC], f32r, kind="ExternalInput").ap()
    wo_d = nc.dram_tensor("wo", [DPC, D], f32r, kind="ExternalInput").ap()
    out_d = nc.dram_tensor("out", [T, D], f32, kind="ExternalOutput").ap()

    with tile.TileContext(nc) as tc, ExitStack() as top:
        persist = top.enter_context(tc.tile_pool(name="persist", bufs=1))
        qt = persist.tile([128, 4, T], f32r, tag="qt")
        kt = persist.tile([128, 4, T], f32r, tag="kt")
        v_sb = persist.tile([128, NT, HPC * (HD + 1)], f32r, tag="v")

        # ones columns for the softmax-denominator rows of the PV matmul
        # (memset can't emit f32r; go through an f32 scratch + rounding copy)
        ones_f32 = persist.tile([128, NT, 1], f32, tag="ones")
        nc.vector.memset(ones_f32, 1.0)
        for h in range(HPC):
            nc.vector.tensor_copy(
                v_sb[:, :, 65 * h + 64 : 65 * h + 65], ones_f32
            )

        # ---------------- phase 1: QT / KT / V projections ----------------
        with ExitStack() as ph1:
            wpool = ph1.enter_context(tc.tile_pool(name="wpool", bufs=1))
            xpool = ph1.enter_context(tc.tile_pool(name="xpool", bufs=2))
            ps1 = ph1.enter_context(tc.tile_pool(name="ps1", bufs=4, space="PSUM"))

            wq = wpool.tile([128, KT, DPC], f32r, tag="wq")
            wk = wpool.tile([128, KT, DPC], f32r, tag="wk")
            wv = wpool.tile([128, KT, DPC], f32r, tag="wv")
            # split per k-tile so the first matmuls start early; use the
            # gpsimd DMA queue so weight loads overlap the x-chunk loads
            # that flow through the sync queue
            for k in range(KT):
                for w_sb, w_d in ((wv, wv_d), (wq, wq_d), (wk, wk_d)):
                    nc.gpsimd.dma_start(
                        out=w_sb[:, k, :],
                        in_=w_d.rearrange("(k p) d -> p k d", p=128)[:, k, :],
                    )

            xT_r = xT.rearrange("(k p) t -> p k t", p=128)
            XC = 256  # token-chunk width for the projection phase
            for tci in range(T // XC):
                xt = xpool.tile([128, KT, XC], f32r, tag="xt")
                nc.sync.dma_start(
                    out=xt, in_=xT_r[:, :, XC * tci : XC * (tci + 1)]
                )
                for tt4 in range(XC // 128):
                    tt = (XC // 128) * tci + tt4
                    ps = ps1.tile([128, DPC], f32, tag="psv")
                    for k in range(KT):
                        nc.tensor.matmul(
                            ps,
                            r(xt[:, k, 128 * tt4 : 128 * (tt4 + 1)]),
                            r(wv[:, k, :]),
                            start=(k == 0),
                            stop=(k == KT - 1),
                        )
                    nc.vector.tensor_copy(
                        v_sb[:, tt, :].rearrange("p (h c) -> p h c", c=HD + 1)[
                            :, :, 0:HD
                        ],
                        ps.rearrange("p (h c) -> p h c", c=HD),
                    )
                for w_sb, dest in ((wq, qt), (wk, kt)):
                    for dt in range(4):
                        ps = ps1.tile([128, XC], f32, tag="ps1")
                        for k in range(KT):
                            nc.tensor.matmul(
                                ps,
                                r(w_sb[:, k, 128 * dt : 128 * (dt + 1)]),
                                r(xt[:, k, :]),
                                start=(k == 0),
                                stop=(k == KT - 1),
                            )
                        nc.vector.tensor_copy(
                            dest[:, dt, XC * tci : XC * (tci + 1)], ps
                        )

        # ---------------- phase 2: attention ----------------
        # prefetch Wo while attention runs
        wop = top.enter_context(tc.tile_pool(name="wop", bufs=1))
        wo = wop.tile([128, 4, D], f32r, tag="wo")
        nc.sync.dma_start(out=wo, in_=wo_d.rearrange("(c p) o -> p c o", p=128))

        ctx_sb = persist.tile([128, 4, T], f32r, tag="ctx")

        with ExitStack() as ph2:
            expp = ph2.enter_context(tc.tile_pool(name="expp", bufs=3))
            smallp = ph2.enter_context(tc.tile_pool(name="smallp", bufs=2))
            pss = ph2.enter_context(tc.tile_pool(name="pss", bufs=2, space="PSUM"))
            psc = ph2.enter_context(tc.tile_pool(name="psc", bufs=2, space="PSUM"))

            def normalize(ctx_ps, h, c):
                """Move ctx out of PSUM and divide rows 0..63 by row 64,
                without any long-latency op on the inter-head critical path.
                The reciprocal of the [1, 1024] sums row would take ~8 cycles
                per element on a single DVE lane; spread it over 128 lanes
                with a pair of tiny SBUF->SBUF DMA reshapes instead."""
                dq = h // 2
                pr = (h % 2) * 64
                raw = smallp.tile([65, 1024], f32, tag="raw")
                nc.vector.tensor_copy(raw, ctx_ps)
                sp = smallp.tile([128, 8], f32, tag="sp")
                nc.sync.dma_start(out=sp, in_=raw[64:65, :])
                rp = smallp.tile([128, 8], f32, tag="rp")
                nc.vector.reciprocal(rp, sp)
                recip = smallp.tile([1, 1024], f32, tag="recip")
                nc.sync.dma_start(out=recip, in_=rp)
                bc = smallp.tile([64, 1024], f32, tag="bc")
                nc.gpsimd.partition_broadcast(bc, recip)
                nc.vector.tensor_mul(
                    ctx_sb[pr : pr + 64, dq, 1024 * c : 1024 * (c + 1)],
                    raw[0:64, :],
                    bc,
                )

            for m in range(4):  # head pair (2m, 2m+1) shares qt/kt tile m
                for c in range(2):
                    ctx_pair = [
                        psc.tile([65, 1024], f32, tag="ctxps", name=f"ctxps_{m}_{c}_{i}")
                        for i in range(2)
                    ]
                    for t in range(8 * c + 8):
                        i0 = max(128 * t, 1024 * c)
                        ext = 1024 * (c + 1) - i0
                        for half in range(2):
                            h = 2 * m + half
                            pr = half * 64
                            s_ps = pss.tile([128, 1024], f32, tag="sps")
                            col = 0
                            while col < ext:
                                w = min(512, ext - col)
                                nc.tensor.matmul(
                                    s_ps[:, col : col + w],
                                    kt[pr : pr + 64, m, 128 * t : 128 * (t + 1)],
                                    qt[pr : pr + 64, m, i0 + col : i0 + col + w],
                                    start=True,
                                    stop=True,
                                )
                                col += w
                            es = expp.tile([128, 1024], f32r, tag="es")
                            nc.scalar.activation(
                                es[:, 0:ext], s_ps[:, 0:ext], Exp, scale=0.125
                            )
                            if i0 == 128 * t:
                                # keep element iff free_idx - partition_idx >= 0
                                nc.gpsimd.affine_select(
                                    out=es[:, 0:128],
                                    in_=es[:, 0:128],
                                    compare_op=mybir.AluOpType.is_ge,
                                    fill=0.0,
                                    base=0,
                                    pattern=[[1, 128]],
                                    channel_multiplier=-1,
                                )
                            for s in range(2):
                                cs = 1024 * c + 512 * s
                                lo = max(i0, cs)
                                hi = cs + 512
                                if lo >= hi:
                                    continue
                                nc.tensor.matmul(
                                    ctx_pair[half][:, lo - 1024 * c : hi - 1024 * c],
                                    v_sb[:, t, 65 * h : 65 * h + 65],
                                    es[:, lo - i0 : hi - i0],
                                    start=(t == 0),
                                    stop=(t == hi // 128 - 1),
                                )
                    for half in range(2):
                        normalize(ctx_pair[half], 2 * m + half, c)

        # ---------------- phase 3: output projection ----------------
        with ExitStack() as ph3:
            outp = ph3.enter_context(tc.tile_pool(name="outp", bufs=6))
            pso = ph3.enter_context(tc.tile_pool(name="pso", bufs=4, space="PSUM"))
            for tt in range(NT):
                for oc in range(2):
                    ps = pso.tile([128, 512], f32, tag="pso")
                    for ct in range(4):
                        nc.tensor.matmul(
                            ps,
                            r(ctx_sb[:, ct, 128 * tt : 128 * (tt + 1)]),
                            r(wo[:, ct, 512 * oc : 512 * (oc + 1)]),
                            start=(ct == 0),
                            stop=(ct == 3),
                        )
                    ot = outp.tile([128, 512], f32, tag="ot")
                    nc.vector.tensor_copy(ot, ps)
                    nc.sync.dma_start(
                        out=out_d[
                            128 * tt : 128 * (tt + 1), 512 * oc : 512 * (oc + 1)
                        ],
                        in_=ot,
                    )

    nc.compile()
    return nc


def _get_program():
    if "nc" not in _CACHE:
        _CACHE["nc"] = _build_program()
    return _CACHE["nc"]


def make_in_maps(x, Wq, Wk, Wv, Wo):
    in_maps = []
    for core in range(NCORES):
        b, hg = core // 2, core % 2
        sl = slice(DPC * hg, DPC * (hg + 1))
        in_maps.append(
            {
                "xT": np.ascontiguousarray(x[b].T),
                "wq": np.ascontiguousarray(Wq[:, sl]),
                "wk": np.ascontiguousarray(Wk[:, sl]),
                "wv": np.ascontiguousarray(Wv[:, sl]),
                "wo": np.ascontiguousarray(Wo[sl, :]),
            }
        )
    return in_maps


def kernel(x, Wq, Wk, Wv, Wo, bo):
    global LAST_RESULTS
    from concourse.bass_utils import run_bass_kernel_spmd

    x = np.asarray(x, dtype=np.float32)
    nc = _get_program()
    in_maps = make_in_maps(
        x,
        np.asarray(Wq, np.float32),
        np.asarray(Wk, np.float32),
        np.asarray(Wv, np.float32),
        np.asarray(Wo, np.float32),
    )
    res = run_bass_kernel_spmd(
        nc,
        in_maps,
        list(range(NCORES)),
        trace=bool(int(os.environ.get("KERNEL_TRACE", "0"))),
    )
    LAST_RESULTS = res
    bo = np.asarray(bo, np.float32)
    out = np.empty((B, T, D), np.float32)
    for b in range(B):
        out[b] = res.results[2 * b]["out"] + res.results[2 * b + 1]["out"] + bo
    return out



# revision 3
# speedup vs baseline: 1.0599x; 1.0599x over previous
"""Multi-head causal attention (B=4, T=2048, D=1024, H=16, hd=64) on 8 trn2 cores.

Sharding: core = (batch, head_group): 4 batches x 2 head-groups of 8 heads.
Each core computes its batch's attention for its 8 heads plus the partial
output projection; the host sums the two head-group partials per batch and
adds the output bias.

All data bf16 (host-cast); PSUM accumulation fp32. The kernel is a software
pipeline over head-pairs m=0..3: attention for pair m runs with the QK/V
projections for pair m+1 interleaved into its tile loop (as closures popped
every few iterations) so the tensor engine never idles while the scalar
engine computes the softmax exps. Layouts:

  xsb  [128, k, tok]        x^T, k = 8 contraction tiles of 128 features
  qt/kt[128, m, tok]        partitions = half*64 + hd for head pair m
  v_sb [128, tt, h, 65]     token-partition V, col 64 = ones (denominator)
  es   [128, 2, 512]        exp(S^T) for both heads of the pair at once
  ctx  [65, 1024] PSUM      row 64 = softmax denominator via ones column
"""

import os
import sys

sys.path.insert(0, "/opt/trn_rl_repo")

import numpy as np

B = 4
T = 2048
D = 1024
H = 16
HD = 64
NCORES = 8
HPC = 8          # heads per core
DPC = HPC * HD   # 512
KT = D // 128    # 8 k-tiles
NT = T // 128    # 16 token tiles
M = 4            # head pairs per core

_CACHE = {}
LAST_RESULTS = None


def _build_program():
    from contextlib import ExitStack

    import concourse.bass as bass
    import concourse.tile as tile
    from concourse import bacc, mybir

    f32 = mybir.dt.float32
    bf16 = mybir.dt.bfloat16
    Exp = mybir.ActivationFunctionType.Exp

    nc = bacc.Bacc(
        "TRN2", target_bir_lowering=False, debug=False, num_devices=NCORES
    )
    # host-prepacked layouts (see make_in_maps)
    x_d = nc.dram_tensor("x", [4 * 128, KT * 512], bf16, kind="ExternalInput").ap()
    wq_d = nc.dram_tensor("wq", [128, M * KT * 128], bf16, kind="ExternalInput").ap()
    wk_d = nc.dram_tensor("wk", [128, M * KT * 128], bf16, kind="ExternalInput").ap()
    wv_d = nc.dram_tensor("wv", [128, M * KT * 128], bf16, kind="ExternalInput").ap()
    wo_d = nc.dram_tensor("wo", [128, M * D], bf16, kind="ExternalInput").ap()
    out_d = nc.dram_tensor("out", [T, D], bf16, kind="ExternalOutput").ap()

    with tile.TileContext(nc) as tc, ExitStack() as top:
        persist = top.enter_context(tc.tile_pool(name="persist", bufs=1))
        xsb = persist.tile([128, KT, T], bf16, tag="xsb")
        wqs = persist.tile([128, M, KT, 128], bf16, tag="wqs")
        wks = persist.tile([128, M, KT, 128], bf16, tag="wks")
        wvs = persist.tile([128, M, KT, 128], bf16, tag="wvs")
        wos = persist.tile([128, M, D], bf16, tag="wos")
        qt = persist.tile([128, M, T], bf16, tag="qt")
        kt = persist.tile([128, M, T], bf16, tag="kt")
        v_sb = persist.tile([128, NT, HPC, HD + 1], bf16, tag="v")
        ctx_sb = persist.tile([128, M, T], bf16, tag="ctx")
        tri2 = persist.tile([128, 2, 128], bf16, tag="tri2")

        # ones columns feed the softmax-denominator row of the PV matmul
        nc.vector.memset(v_sb[:, :, :, HD : HD + 1], 1.0)
        # causal keep-mask for the 128-wide diagonal block, one copy per
        # half-plane: tri2[p, :, q] = 1 if q >= p else 0
        nc.vector.memset(tri2, 1.0)
        for i in range(2):
            nc.gpsimd.affine_select(
                out=tri2[:, i, :],
                in_=tri2[:, i, :],
                compare_op=mybir.AluOpType.is_ge,
                fill=0.0,
                base=0,
                pattern=[[1, 128]],
                channel_multiplier=-1,
            )

        # weights on the gpsimd DMA queue, m-major so pair 0 lands first
        wq_r = wq_d.rearrange("p (m k d) -> p m k d", m=M, k=KT)
        wk_r = wk_d.rearrange("p (m k d) -> p m k d", m=M, k=KT)
        wv_r = wv_d.rearrange("p (m k d) -> p m k d", m=M, k=KT)
        for m in range(M):
            for w_sb, w_r in ((wqs, wq_r), (wks, wk_r), (wvs, wv_r)):
                nc.gpsimd.dma_start(out=w_sb[:, m], in_=w_r[:, m])
        nc.gpsimd.dma_start(
            out=wos, in_=wo_d.rearrange("p (m o) -> p m o", m=M)
        )
        # x on the sync queue, token-chunk-major / k-minor so the first
        # projection's contraction inputs arrive in consumption order
        for ci in range(4):
            for k in range(KT):
                nc.sync.dma_start(
                    out=xsb[:, k, 512 * ci : 512 * (ci + 1)],
                    in_=x_d[128 * ci : 128 * (ci + 1), 512 * k : 512 * (k + 1)],
                )

        pss = top.enter_context(tc.tile_pool(name="pss", bufs=2, space="PSUM"))
        psc = top.enter_context(tc.tile_pool(name="psc", bufs=2, space="PSUM"))
        expp = top.enter_context(tc.tile_pool(name="expp", bufs=3))
        smallp = top.enter_context(tc.tile_pool(name="smallp", bufs=2))
        outp = top.enter_context(tc.tile_pool(name="outp", bufs=3))

        def qk_chunk(m, ci):
            """Project Q^T and K^T for head pair m over tokens [512ci, 512ci+512)."""
            ps = pss.tile([128, 2, 512], f32, tag="pp", name=f"pqk_{m}_{ci}")
            for j, (w_sb, dest) in enumerate(((wqs, qt), (wks, kt))):
                for k in range(KT):
                    nc.tensor.matmul(
                        ps[:, j, :],
                        w_sb[:, m, k, :],
                        xsb[:, k, 512 * ci : 512 * (ci + 1)],
                        start=(k == 0),
                        stop=(k == KT - 1),
                    )
                nc.vector.tensor_copy(
                    dest[:, m, 512 * ci : 512 * (ci + 1)], ps[:, j, :]
                )

        def v_pair(m, tp):
            """Project V for head pair m over token tiles 2tp, 2tp+1."""
            ps = pss.tile([128, 2, 512], f32, tag="pp", name=f"pv_{m}_{tp}")
            for j in range(2):
                tt = 2 * tp + j
                for k in range(KT):
                    nc.tensor.matmul(
                        ps[:, j, 0:128],
                        xsb[:, k, 128 * tt : 128 * (tt + 1)],
                        wvs[:, m, k, :],
                        start=(k == 0),
                        stop=(k == KT - 1),
                    )
                nc.vector.tensor_copy(
                    v_sb[:, tt, 2 * m : 2 * m + 2, 0:HD],
                    ps[:, j, 0:128].rearrange("p (h c) -> p h c", c=HD),
                )

        def out_tile(tt):
            """Output projection for token tile tt (all four head pairs)."""
            ps = pss.tile([128, 2, 512], f32, tag="pp", name=f"po_{tt}")
            for oc in range(2):
                for ct in range(M):
                    nc.tensor.matmul(
                        ps[:, oc, :],
                        ctx_sb[:, ct, 128 * tt : 128 * (tt + 1)],
                        wos[:, ct, 512 * oc : 512 * (oc + 1)],
                        start=(ct == 0),
                        stop=(ct == M - 1),
                    )
            ot = outp.tile([128, 1024], bf16, tag="ot", name=f"ot_{tt}")
            nc.vector.tensor_copy(
                ot.rearrange("p (a b) -> p a b", a=2), ps
            )
            nc.sync.dma_start(
                out=out_d[128 * tt : 128 * (tt + 1), :], in_=ot
            )

        def normalize(ctx_ps, m, c, half):
            """ctx_sb[...] = ctx_ps[0:64] / ctx_ps[64] (the denominator row).
            The copy to SBUF frees the PSUM bank fast; the reciprocal is
            spread over 128 lanes via two tiny DMA reshapes; the broadcast
            and final multiply run on gpsimd to keep the vector queue free
            for the next attention chunk's mask ops."""
            raw = smallp.tile([65, 1024], f32, tag="raw", name=f"raw_{m}_{c}_{half}")
            nc.vector.tensor_copy(raw, ctx_ps)
            sp8 = smallp.tile([128, 8], f32, tag="sp8", name=f"sp8_{m}_{c}_{half}")
            nc.sync.dma_start(out=sp8, in_=raw[64:65, :])
            rp8 = smallp.tile([128, 8], f32, tag="rp8", name=f"rp8_{m}_{c}_{half}")
            nc.vector.reciprocal(rp8, sp8)
            recip = smallp.tile([1, 1024], f32, tag="recip", name=f"rc_{m}_{c}_{half}")
            nc.sync.dma_start(out=recip, in_=rp8)
            bc = smallp.tile([64, 1024], f32, tag="bc", name=f"bc_{m}_{c}_{half}")
            nc.gpsimd.partition_broadcast(bc, recip)
            nc.gpsimd.tensor_mul(
                ctx_sb[64 * half : 64 * half + 64, m, 1024 * c : 1024 * (c + 1)],
                raw[0:64, :],
                bc,
            )

        # prologue: projections for pair 0
        for ci in range(4):
            qk_chunk(0, ci)
        for tp in range(NT // 2):
            v_pair(0, tp)

        # main pipeline over head pairs
        for m in range(M):
            for c in range(2):
                if m < M - 1:
                    if c == 0:
                        closures = [
                            (lambda mm=m + 1, cc=ci: qk_chunk(mm, cc))
                            for ci in range(4)
                        ]
                    else:
                        closures = [
                            (lambda mm=m + 1, pp=tp: v_pair(mm, pp))
                            for tp in range(NT // 2)
                        ]
                else:
                    closures = (
                        [(lambda t=j: out_tile(t)) for j in range(8)]
                        if c == 1
                        else []
                    )
                ctxp = [
                    psc.tile([65, 1024], f32, tag="ctx", name=f"ctx_{m}_{c}_{i}")
                    for i in range(2)
                ]
                it = 0
                for t in range(8 * c + 8):
                    j0 = 128 * t
                    for qc in (1024 * c, 1024 * c + 512):
                        if qc + 512 <= j0:
                            continue
                        col0 = max(j0, qc)
                        w = qc + 512 - col0
                        sp = pss.tile(
                            [128, 2, 512], f32, tag="pp", name=f"sp_{m}_{c}_{t}_{qc}"
                        )
                        for half in range(2):
                            pr = 64 * half
                            nc.tensor.matmul(
                                sp[:, half, 0:w],
                                kt[pr : pr + 64, m, j0 : j0 + 128],
                                qt[pr : pr + 64, m, col0 : col0 + w],
                                start=True,
                                stop=True,
                            )
                        es = expp.tile(
                            [128, 2, 512], bf16, tag="es", name=f"es_{m}_{c}_{t}_{qc}"
                        )
                        nc.scalar.activation(
                            es[:, :, 0:w], sp[:, :, 0:w], Exp, scale=0.125
                        )
                        if col0 == j0:
                            # zero the upper triangle of the diagonal block
                            nc.vector.tensor_mul(
                                es[:, :, 0:128], es[:, :, 0:128], tri2
                            )
                        stop_t = qc // 128 + 3
                        for half in range(2):
                            nc.tensor.matmul(
                                ctxp[half][
                                    :, col0 - 1024 * c : col0 - 1024 * c + w
                                ],
                                v_sb[:, t, 2 * m + half, :],
                                es[:, half, 0:w],
                                start=(t == 0),
                                stop=(t == stop_t),
                            )
                        if it % 3 == 2 and closures:
                            closures.pop(0)()
                        it += 1
                while closures:
                    closures.pop(0)()
                for half in range(2):
                    normalize(ctxp[half], m, c, half)

        # epilogue: output projection for the second token half
        for tt in range(8, NT):
            out_tile(tt)

    nc.compile()
    return nc


def _get_program():
    if "nc" not in _CACHE:
        _CACHE["nc"] = _build_program()
    return _CACHE["nc"]


def make_in_maps(x, Wq, Wk, Wv, Wo):
    import ml_dtypes

    bf16 = ml_dtypes.bfloat16
    in_maps = []
    for core in range(NCORES):
        b, hg = core // 2, core % 2
        sl = slice(DPC * hg, DPC * (hg + 1))
        # x: [ci*128+p, k*512+tok] = x[b].T[k*128+p, 512ci+tok]
        xb = np.ascontiguousarray(x[b].T).astype(bf16)
        xr = (
            xb.reshape(KT, 128, 4, 512).transpose(2, 1, 0, 3).reshape(512, 4096)
        )
        # wq/wk/wv: [p, m*1024 + k*128 + d] = W[k*128+p, m*128+d]
        def pack_w(W):
            return np.ascontiguousarray(
                W[:, sl]
                .reshape(KT, 128, M, 128)
                .transpose(1, 2, 0, 3)
                .reshape(128, M * KT * 128)
            ).astype(bf16)

        # wo: [p, m*1024 + o] = Wo[sl][m*128+p, o]
        wo = np.ascontiguousarray(
            Wo[sl, :].reshape(M, 128, D).transpose(1, 0, 2).reshape(128, M * D)
        ).astype(bf16)
        in_maps.append(
            {
                "x": np.ascontiguousarray(xr),
                "wq": pack_w(Wq),
                "wk": pack_w(Wk),
                "wv": pack_w(Wv),
                "wo": wo,
            }
        )
    return in_maps


def kernel(x, Wq, Wk, Wv, Wo, bo):
    global LAST_RESULTS
    from concourse.bass_utils import run_bass_kernel_spmd

    x = np.asarray(x, dtype=np.float32)
    nc = _get_program()
    in_maps = make_in_maps(
        x,
        np.asarray(Wq, np.float32),
        np.asarray(Wk, np.float32),
        np.asarray(Wv, np.float32),
        np.asarray(Wo, np.float32),
    )
    res = run_bass_kernel_spmd(
        nc,
        in_maps,
        list(range(NCORES)),
        trace=bool(int(os.environ.get("KERNEL_TRACE", "0"))),
    )
    LAST_RESULTS = res
    bo = np.asarray(bo, np.float32)
    out = np.empty((B, T, D), np.float32)
    for b in range(B):
        out[b] = (
            res.results[2 * b]["out"].astype(np.float32)
            + res.results[2 * b + 1]["out"].astype(np.float32)
            + bo
        )
    return out


# revision 4
# speedup vs baseline: 1.1192x; 1.0559x over previous
"""Multi-head causal attention (B=4, T=2048, D=1024, H=16, hd=64) on 8 trn2 cores.

Sharding: core = (batch, head_group): 4 batches x 2 head-groups of 8 heads.
Each core computes its batch's attention for its 8 heads plus the partial
output projection; the host sums the two head-group partials per batch and
adds the output bias.

All data bf16 (host-cast); PSUM accumulation fp32. The kernel is a software
pipeline: attention runs query-chunk-major (qc = 512 queries) over head
pairs m, token-half-major overall, with the QK/V projections for the next
head pair (and later the output projection) interleaved into the attention
loop as closures so the tensor engine never idles (keeping it at its top
p-state) while the scalar engine computes the softmax exps. Layouts:

  xsb  [128, k, tok]        x^T, k = 8 contraction tiles of 128 features
  qt/kt[128, m, tok]        partitions = half*64 + hd for head pair m
  v_sb [128, tt, h, 65]     token-partition V, col 64 = ones (denominator)
  es   [128, 2, 512]        exp(S^T) for both heads of the pair at once
  ctx  [65, 512] PSUM       row 64 = softmax denominator via ones column
"""

import os
import sys

sys.path.insert(0, "/opt/trn_rl_repo")

import numpy as np

B = 4
T = 2048
D = 1024
H = 16
HD = 64
NCORES = 8
HPC = 8          # heads per core
DPC = HPC * HD   # 512
KT = D // 128    # 8 k-tiles
NT = T // 128    # 16 token tiles
M = 4            # head pairs per core

_CACHE = {}
LAST_RESULTS = None


def _build_program():
    from contextlib import ExitStack

    import concourse.bass as bass
    import concourse.tile as tile
    from concourse import bacc, mybir

    f32 = mybir.dt.float32
    bf16 = mybir.dt.bfloat16
    Exp = mybir.ActivationFunctionType.Exp

    nc = bacc.Bacc(
        "TRN2", target_bir_lowering=False, debug=False, num_devices=NCORES
    )
    # host-prepacked layouts (see make_in_maps)
    x_d = nc.dram_tensor("x", [4 * 128, KT * 512], bf16, kind="ExternalInput").ap()
    wq_d = nc.dram_tensor("wq", [128, M * KT * 128], bf16, kind="ExternalInput").ap()
    wk_d = nc.dram_tensor("wk", [128, M * KT * 128], bf16, kind="ExternalInput").ap()
    wv_d = nc.dram_tensor("wv", [128, M * KT * 128], bf16, kind="ExternalInput").ap()
    wo_d = nc.dram_tensor("wo", [128, M * D], bf16, kind="ExternalInput").ap()
    out_d = nc.dram_tensor("out", [T, D], bf16, kind="ExternalOutput").ap()

    with tile.TileContext(nc) as tc, ExitStack() as top:
        persist = top.enter_context(tc.tile_pool(name="persist", bufs=1))
        xsb = persist.tile([128, KT, T], bf16, tag="xsb")
        wqs = persist.tile([128, M, KT, 128], bf16, tag="wqs")
        wks = persist.tile([128, M, KT, 128], bf16, tag="wks")
        wvs = persist.tile([128, M, KT, 128], bf16, tag="wvs")
        wos = persist.tile([128, M, D], bf16, tag="wos")
        qt = persist.tile([128, M, T], bf16, tag="qt")
        kt = persist.tile([128, M, T], bf16, tag="kt")
        v_sb = persist.tile([128, NT, HPC, HD + 1], bf16, tag="v")
        ctx_sb = persist.tile([128, M, T], bf16, tag="ctx")
        tri2 = persist.tile([128, 2, 128], bf16, tag="tri2")

        # ones columns feed the softmax-denominator row of the PV matmul
        nc.vector.memset(v_sb[:, :, :, HD : HD + 1], 1.0)
        # causal keep-mask for the 128-wide diagonal block, one copy per
        # half-plane: tri2[p, :, q] = 1 if q >= p else 0
        nc.vector.memset(tri2, 1.0)
        for i in range(2):
            nc.gpsimd.affine_select(
                out=tri2[:, i, :],
                in_=tri2[:, i, :],
                compare_op=mybir.AluOpType.is_ge,
                fill=0.0,
                base=0,
                pattern=[[1, 128]],
                channel_multiplier=-1,
            )

        # weights on the gpsimd DMA queue, m-major so pair 0 lands first
        wq_r = wq_d.rearrange("p (m k d) -> p m k d", m=M, k=KT)
        wk_r = wk_d.rearrange("p (m k d) -> p m k d", m=M, k=KT)
        wv_r = wv_d.rearrange("p (m k d) -> p m k d", m=M, k=KT)
        for m in range(M):
            for w_sb, w_r in ((wqs, wq_r), (wks, wk_r), (wvs, wv_r)):
                nc.gpsimd.dma_start(out=w_sb[:, m], in_=w_r[:, m])
        nc.gpsimd.dma_start(
            out=wos, in_=wo_d.rearrange("p (m o) -> p m o", m=M)
        )
        # x on the sync queue, one large DMA per 512-token chunk
        for ci in range(4):
            nc.sync.dma_start(
                out=xsb[:, :, 512 * ci : 512 * (ci + 1)],
                in_=x_d[128 * ci : 128 * (ci + 1), :],
            )

        pss = top.enter_context(tc.tile_pool(name="pss", bufs=2, space="PSUM"))
        psc = top.enter_context(tc.tile_pool(name="psc", bufs=4, space="PSUM"))
        expp = top.enter_context(tc.tile_pool(name="expp", bufs=3))
        smallp = top.enter_context(tc.tile_pool(name="smallp", bufs=2))
        outp = top.enter_context(tc.tile_pool(name="outp", bufs=3))

        def qk_chunk(m, ci):
            """Project Q^T and K^T for head pair m over tokens [512ci, 512ci+512)."""
            ps = pss.tile([128, 2, 512], f32, tag="pp", name=f"pqk_{m}_{ci}")
            for j, (w_sb, dest) in enumerate(((wqs, qt), (wks, kt))):
                for k in range(KT):
                    nc.tensor.matmul(
                        ps[:, j, :],
                        w_sb[:, m, k, :],
                        xsb[:, k, 512 * ci : 512 * (ci + 1)],
                        start=(k == 0),
                        stop=(k == KT - 1),
                    )
                nc.vector.tensor_copy(
                    dest[:, m, 512 * ci : 512 * (ci + 1)], ps[:, j, :]
                )

        def v_pair(m, tp):
            """Project V for head pair m over token tiles 2tp, 2tp+1."""
            ps = pss.tile([128, 2, 512], f32, tag="pp", name=f"pv_{m}_{tp}")
            for j in range(2):
                tt = 2 * tp + j
                for k in range(KT):
                    nc.tensor.matmul(
                        ps[:, j, 0:128],
                        xsb[:, k, 128 * tt : 128 * (tt + 1)],
                        wvs[:, m, k, :],
                        start=(k == 0),
                        stop=(k == KT - 1),
                    )
                nc.vector.tensor_copy(
                    v_sb[:, tt, 2 * m : 2 * m + 2, 0:HD],
                    ps[:, j, 0:128].rearrange("p (h c) -> p h c", c=HD),
                )

        def out_tile(tt):
            """Output projection for token tile tt (all four head pairs)."""
            ps = pss.tile([128, 2, 512], f32, tag="pp", name=f"po_{tt}")
            for oc in range(2):
                for ct in range(M):
                    nc.tensor.matmul(
                        ps[:, oc, :],
                        ctx_sb[:, ct, 128 * tt : 128 * (tt + 1)],
                        wos[:, ct, 512 * oc : 512 * (oc + 1)],
                        start=(ct == 0),
                        stop=(ct == M - 1),
                    )
            ot = outp.tile([128, 1024], bf16, tag="ot", name=f"ot_{tt}")
            nc.vector.tensor_copy(
                ot.rearrange("p (a b) -> p a b", a=2), ps
            )
            nc.sync.dma_start(
                out=out_d[128 * tt : 128 * (tt + 1), :], in_=ot
            )

        def normalize(ctxp, m, qc):
            """ctx_sb[.., qc:qc+512] = ctxp[h][0:64] / ctxp[h][64] for both
            halves. PSUM-freeing copies go first (they gate the attention
            pipeline via the psc ring); the reciprocal is spread over 128
            lanes via two small DMA reshapes issued from the gpsimd queue
            (keeping the sync queue free for x/out traffic); broadcast and
            the final multiplies run on gpsimd, off every critical path."""
            tag = f"{m}_{qc}"
            raw = smallp.tile([65, 2, 512], f32, tag="raw", name=f"raw_{tag}")
            for h in range(2):
                nc.vector.tensor_copy(raw[:, h, :], ctxp[h])
            sp8 = smallp.tile([128, 8], f32, tag="sp8", name=f"sp8_{tag}")
            nc.gpsimd.dma_start(out=sp8, in_=raw[64:65, :, :])
            rp8 = smallp.tile([128, 8], f32, tag="rp8", name=f"rp8_{tag}")
            nc.vector.reciprocal(rp8, sp8)
            recip = smallp.tile([1, 2, 512], f32, tag="recip", name=f"rc_{tag}")
            nc.gpsimd.dma_start(out=recip, in_=rp8)
            bc = smallp.tile([64, 2, 512], f32, tag="bc", name=f"bc_{tag}")
            nc.gpsimd.partition_broadcast(bc, recip)
            for h in range(2):
                nc.gpsimd.tensor_mul(
                    ctx_sb[64 * h : 64 * h + 64, m, qc : qc + 512],
                    raw[0:64, h, :],
                    bc[:, h, :],
                )

        def attn(m, qc, closures, pop_every):
            """Causal attention for head pair m, queries [qc, qc+512)."""
            ctxp = [
                psc.tile([65, 512], f32, tag="ctx", name=f"ctx_{m}_{qc}_{i}")
                for i in range(2)
            ]
            tmax = qc // 128 + 3
            for t in range(tmax + 1):
                j0 = 128 * t
                col0 = max(j0, qc)
                w = qc + 512 - col0
                sp = pss.tile([128, 2, 512], f32, tag="pp", name=f"sp_{m}_{qc}_{t}")
                for half in range(2):
                    pr = 64 * half
                    nc.tensor.matmul(
                        sp[:, half, 0:w],
                        kt[pr : pr + 64, m, j0 : j0 + 128],
                        qt[pr : pr + 64, m, col0 : col0 + w],
                        start=True,
                        stop=True,
                    )
                es = expp.tile(
                    [128, 2, 512], bf16, tag="es", name=f"es_{m}_{qc}_{t}"
                )
                nc.scalar.activation(
                    es[:, :, 0:w], sp[:, :, 0:w], Exp, scale=0.125
                )
                if col0 == j0:
                    # zero the upper triangle of the diagonal block
                    nc.vector.tensor_mul(es[:, :, 0:128], es[:, :, 0:128], tri2)
                for half in range(2):
                    nc.tensor.matmul(
                        ctxp[half][:, col0 - qc : col0 - qc + w],
                        v_sb[:, t, 2 * m + half, :],
                        es[:, half, 0:w],
                        start=(t == 0),
                        stop=(t == tmax),
                    )
                if t % pop_every == pop_every - 1 and closures:
                    closures.pop(0)()
            normalize(ctxp, m, qc)

        # prologue: projections for pair 0
        for ci in range(4):
            qk_chunk(0, ci)
        for tp in range(NT // 2):
            v_pair(0, tp)

        # main pipeline: token-half-major, head pair m inner, 512-query
        # chunks innermost
        for c in range(2):
            for m in range(M):
                if c == 0 and m < M - 1:
                    closures = [
                        (lambda mm=m + 1, cc=ci: qk_chunk(mm, cc))
                        for ci in range(4)
                    ] + [
                        (lambda mm=m + 1, pp=tp: v_pair(mm, pp))
                        for tp in range(NT // 2)
                    ]
                    pop_every = 1
                elif c == 1 and m >= 1:
                    # first-half output tiles: ctx for tokens [0,1024) is
                    # complete once (c=0, m=3) has normalized
                    base = 3 * (m - 1)
                    nout = 3 if m < 3 else 2
                    closures = [
                        (lambda t=base + j: out_tile(t)) for j in range(nout)
                    ]
                    pop_every = 9
                else:
                    closures = []
                    pop_every = 1000
                for qc in (1024 * c, 1024 * c + 512):
                    attn(m, qc, closures, pop_every)
                while closures:
                    closures.pop(0)()

        # epilogue: remaining output tiles
        for tt in range(8, NT):
            out_tile(tt)

    nc.compile()
    return nc


def _get_program():
    if "nc" not in _CACHE:
        _CACHE["nc"] = _build_program()
    return _CACHE["nc"]


def make_in_maps(x, Wq, Wk, Wv, Wo):
    import ml_dtypes

    bf16 = ml_dtypes.bfloat16
    in_maps = []
    for core in range(NCORES):
        b, hg = core // 2, core % 2
        sl = slice(DPC * hg, DPC * (hg + 1))
        # x: [ci*128+p, k*512+tok] = x[b].T[k*128+p, 512ci+tok]
        xb = np.ascontiguousarray(x[b].T).astype(bf16)
        xr = (
            xb.reshape(KT, 128, 4, 512).transpose(2, 1, 0, 3).reshape(512, 4096)
        )
        # wq/wk/wv: [p, m*1024 + k*128 + d] = W[k*128+p, m*128+d]
        def pack_w(W):
            return np.ascontiguousarray(
                W[:, sl]
                .reshape(KT, 128, M, 128)
                .transpose(1, 2, 0, 3)
                .reshape(128, M * KT * 128)
            ).astype(bf16)

        # wo: [p, m*1024 + o] = Wo[sl][m*128+p, o]
        wo = np.ascontiguousarray(
            Wo[sl, :].reshape(M, 128, D).transpose(1, 0, 2).reshape(128, M * D)
        ).astype(bf16)
        in_maps.append(
            {
                "x": np.ascontiguousarray(xr),
                "wq": pack_w(Wq),
                "wk": pack_w(Wk),
                "wv": pack_w(Wv),
                "wo": wo,
            }
        )
    return in_maps


def kernel(x, Wq, Wk, Wv, Wo, bo):
    global LAST_RESULTS
    from concourse.bass_utils import run_bass_kernel_spmd

    x = np.asarray(x, dtype=np.float32)
    nc = _get_program()
    in_maps = make_in_maps(
        x,
        np.asarray(Wq, np.float32),
        np.asarray(Wk, np.float32),
        np.asarray(Wv, np.float32),
        np.asarray(Wo, np.float32),
    )
    res = run_bass_kernel_spmd(
        nc,
        in_maps,
        list(range(NCORES)),
        trace=bool(int(os.environ.get("KERNEL_TRACE", "0"))),
    )
    LAST_RESULTS = res
    bo = np.asarray(bo, np.float32)
    out = np.empty((B, T, D), np.float32)
    for b in range(B):
        out[b] = (
            res.results[2 * b]["out"].astype(np.float32)
            + res.results[2 * b + 1]["out"].astype(np.float32)
            + bo
        )
    return out


# revision 8
# speedup vs baseline: 1.2874x; 1.1503x over previous
"""Multi-head causal attention (B=4, T=2048, D=1024, H=16, hd=64) on 8 trn2 cores.

Sharding: core = (batch, head_group): 4 batches x 2 head-groups of 8 heads.
Each core computes its batch's attention for its 8 heads plus the partial
output projection; the host sums the two head-group partials per batch and
adds the output bias.

All data bf16 (host-cast); PSUM accumulation fp32. The kernel is a software
pipeline: attention runs query-chunk-major (qc = 512 queries) over head
pairs m, token-half-major overall, with the QK/V projections for the next
head pair (and later the output projection) interleaved into the attention
loop as closures so the tensor engine never idles while the scalar engine
computes the softmax exps (the per-iteration critical resource). The two
S-matmul halves of a head pair run concurrently on disjoint PE row-halves.
Softmax normalization is batched per (token-half, pair): one reciprocal
lane-spread round-trip for all four [65,512] context accumulators, with the
latency-bearing stages deferred into the next attention loop. Layouts:

  xsb  [128, k, tok]        x^T, k = 8 contraction tiles of 128 features
  qt/kt[128, m, tok]        partitions = half*64 + hd for head pair m
  v_sb [128, tt, h, 65]     token-partition V, col 64 = ones (denominator)
  es   [128, 2, 512]        exp(S^T) for both heads of the pair at once
  ctx  [65, 512] PSUM       row 64 = softmax denominator via ones column
"""

import os
import sys

sys.path.insert(0, "/opt/trn_rl_repo")

import numpy as np

B = 4
T = 2048
D = 1024
H = 16
HD = 64
NCORES = 8
HPC = 8          # heads per core
DPC = HPC * HD   # 512
KT = D // 128    # 8 k-tiles
NT = T // 128    # 16 token tiles
M = 4            # head pairs per core

_CACHE = {}
LAST_RESULTS = None


def _build_program():
    from contextlib import ExitStack

    import concourse.bass as bass
    import concourse.tile as tile
    from concourse import bacc, mybir

    f32 = mybir.dt.float32
    bf16 = mybir.dt.bfloat16
    Exp = mybir.ActivationFunctionType.Exp

    nc = bacc.Bacc(
        "TRN2", target_bir_lowering=False, debug=False, num_devices=NCORES
    )
    # host-prepacked layouts (see make_in_maps)
    x_d = nc.dram_tensor("x", [4 * 128, KT * 512], bf16, kind="ExternalInput").ap()
    wq_d = nc.dram_tensor("wq", [128, M * KT * 128], bf16, kind="ExternalInput").ap()
    wk_d = nc.dram_tensor("wk", [128, M * KT * 128], bf16, kind="ExternalInput").ap()
    wv_d = nc.dram_tensor("wv", [128, M * KT * 128], bf16, kind="ExternalInput").ap()
    wo_d = nc.dram_tensor("wo", [128, M * D], bf16, kind="ExternalInput").ap()
    out_d = nc.dram_tensor("out", [T, D], bf16, kind="ExternalOutput").ap()

    with tile.TileContext(nc) as tc, ExitStack() as top:
        persist = top.enter_context(tc.tile_pool(name="persist", bufs=1))
        xsb = persist.tile([128, KT, T], bf16, tag="xsb")
        wqs = persist.tile([128, M, KT, 128], bf16, tag="wqs")
        wks = persist.tile([128, M, KT, 128], bf16, tag="wks")
        wvs = persist.tile([128, M, KT, 128], bf16, tag="wvs")
        wos = persist.tile([128, M, D], bf16, tag="wos")
        qt = persist.tile([128, M, T], bf16, tag="qt")
        kt = persist.tile([128, M, T], bf16, tag="kt")
        v_sb = persist.tile([128, NT, HPC, HD + 1], bf16, tag="v")
        ctx_sb = persist.tile([128, M, T], bf16, tag="ctx")
        tri2 = persist.tile([128, 2, 128], bf16, tag="tri2")

        # ones columns feed the softmax-denominator row of the PV matmul
        nc.vector.memset(v_sb[:, :, :, HD : HD + 1], 1.0)
        # causal keep-mask for the 128-wide diagonal block, one copy per
        # half-plane: tri2[p, :, q] = 1 if q >= p else 0
        nc.vector.memset(tri2, 1.0)
        for i in range(2):
            nc.gpsimd.affine_select(
                out=tri2[:, i, :],
                in_=tri2[:, i, :],
                compare_op=mybir.AluOpType.is_ge,
                fill=0.0,
                base=0,
                pattern=[[1, 128]],
                channel_multiplier=-1,
            )

        # weights on the gpsimd DMA queue, m-major so pair 0 lands first
        wq_r = wq_d.rearrange("p (m k d) -> p m k d", m=M, k=KT)
        wk_r = wk_d.rearrange("p (m k d) -> p m k d", m=M, k=KT)
        wv_r = wv_d.rearrange("p (m k d) -> p m k d", m=M, k=KT)
        for m in range(M):
            for w_sb, w_r in ((wqs, wq_r), (wks, wk_r), (wvs, wv_r)):
                nc.gpsimd.dma_start(out=w_sb[:, m], in_=w_r[:, m])
        nc.gpsimd.dma_start(
            out=wos, in_=wo_d.rearrange("p (m o) -> p m o", m=M)
        )
        # x on the sync queue: first chunk split per k-tile so the first
        # projection's contraction inputs arrive in consumption order
        for k in range(KT):
            nc.sync.dma_start(
                out=xsb[:, k, 0:512], in_=x_d[0:128, 512 * k : 512 * (k + 1)]
            )
        for ci in range(1, 4):
            nc.sync.dma_start(
                out=xsb[:, :, 512 * ci : 512 * (ci + 1)],
                in_=x_d[128 * ci : 128 * (ci + 1), :],
            )

        pss = top.enter_context(tc.tile_pool(name="pss", bufs=2, space="PSUM"))
        psc = top.enter_context(tc.tile_pool(name="psc", bufs=4, space="PSUM"))
        expp = top.enter_context(tc.tile_pool(name="expp", bufs=3))
        smallp = top.enter_context(tc.tile_pool(name="smallp", bufs=2))
        outp = top.enter_context(tc.tile_pool(name="outp", bufs=3))

        def qk_chunk(m, ci):
            """Project Q^T and K^T for head pair m over tokens [512ci, 512ci+512)."""
            ps = pss.tile([128, 2, 512], f32, tag="pp", name=f"pqk_{m}_{ci}")
            for j, (w_sb, dest) in enumerate(((wqs, qt), (wks, kt))):
                for k in range(KT):
                    nc.tensor.matmul(
                        ps[:, j, :],
                        w_sb[:, m, k, :],
                        xsb[:, k, 512 * ci : 512 * (ci + 1)],
                        start=(k == 0),
                        stop=(k == KT - 1),
                    )
                nc.vector.tensor_copy(
                    dest[:, m, 512 * ci : 512 * (ci + 1)], ps[:, j, :]
                )

        def v_pair(m, tp):
            """Project V for head pair m over token tiles 2tp, 2tp+1."""
            ps = pss.tile([128, 2, 512], f32, tag="pp", name=f"pv_{m}_{tp}")
            for j in range(2):
                tt = 2 * tp + j
                for k in range(KT):
                    nc.tensor.matmul(
                        ps[:, j, 0:128],
                        xsb[:, k, 128 * tt : 128 * (tt + 1)],
                        wvs[:, m, k, :],
                        start=(k == 0),
                        stop=(k == KT - 1),
                    )
                nc.vector.tensor_copy(
                    v_sb[:, tt, 2 * m : 2 * m + 2, 0:HD],
                    ps[:, j, 0:128].rearrange("p (h c) -> p h c", c=HD),
                )

        def out_tile(tt):
            """Output projection for token tile tt (all four head pairs)."""
            ps = pss.tile([128, 2, 512], f32, tag="pp", name=f"po_{tt}")
            for oc in range(2):
                for ct in range(M):
                    nc.tensor.matmul(
                        ps[:, oc, :],
                        ctx_sb[:, ct, 128 * tt : 128 * (tt + 1)],
                        wos[:, ct, 512 * oc : 512 * (oc + 1)],
                        start=(ct == 0),
                        stop=(ct == M - 1),
                    )
            ot = outp.tile([128, 1024], bf16, tag="ot", name=f"ot_{tt}")
            nc.vector.tensor_copy(
                ot.rearrange("p (a b) -> p a b", a=2), ps
            )
            nc.sync.dma_start(
                out=out_d[128 * tt : 128 * (tt + 1), :], in_=ot
            )

        def norm_finish(raw, m, c):
            """Deferred stage of the batched softmax normalization for
            (token half c, pair m): spread the 2048 denominators over 128
            lanes via two small DMA reshapes (a [1,2048] reciprocal on one
            DVE lane would be ~8 cycles/elem), broadcast the reciprocals,
            and scale the context. Broadcast+multiplies live on gpsimd,
            which has no PE-critical work queued."""
            tag = f"{m}_{c}"
            sp16 = smallp.tile([128, 16], f32, tag="sp16", name=f"sp_{tag}")
            nc.sync.dma_start(out=sp16, in_=raw[64:65, :, :, :])
            rp16 = smallp.tile([128, 16], f32, tag="rp16", name=f"rp_{tag}")
            nc.vector.reciprocal(rp16, sp16)
            recip = smallp.tile([1, 2, 2, 512], f32, tag="recip", name=f"rc_{tag}")
            nc.sync.dma_start(out=recip, in_=rp16)
            bcr = smallp.tile([64, 2, 2, 512], f32, tag="bcr", name=f"bc_{tag}")
            nc.gpsimd.partition_broadcast(bcr, recip)
            for h in range(2):
                nc.gpsimd.tensor_mul(
                    ctx_sb[
                        64 * h : 64 * h + 64, m, 1024 * c : 1024 * (c + 1)
                    ].rearrange("p (a b) -> p a b", a=2),
                    raw[0:64, :, h, :],
                    bcr[:, :, h, :],
                )

        pending = []  # deferred norm_finish closures

        def attn(m, c, qi, raw, closures, pop_iters, it_base):
            """Causal attention for head pair m, queries [qc, qc+512)."""
            qc = 1024 * c + 512 * qi
            ctxp = [
                psc.tile([65, 512], f32, tag="ctx", name=f"ctx_{m}_{qc}_{i}")
                for i in range(2)
            ]
            tmax = qc // 128 + 3
            for t in range(tmax + 1):
                j0 = 128 * t
                col0 = max(j0, qc)
                w = qc + 512 - col0
                sp = pss.tile([128, 2, 512], f32, tag="pp", name=f"sp_{m}_{qc}_{t}")
                for half in range(2):
                    pr = 64 * half
                    nc.tensor.matmul(
                        sp[:, half, 0:w],
                        kt[pr : pr + 64, m, j0 : j0 + 128],
                        qt[pr : pr + 64, m, col0 : col0 + w],
                        start=True,
                        stop=True,
                    )
                es = expp.tile(
                    [128, 2, 512], bf16, tag="es", name=f"es_{m}_{qc}_{t}"
                )
                nc.scalar.activation(
                    es[:, :, 0:w], sp[:, :, 0:w], Exp, scale=0.125
                )
                if col0 == j0:
                    # zero the upper triangle of the diagonal block
                    nc.vector.tensor_mul(es[:, :, 0:128], es[:, :, 0:128], tri2)
                for half in range(2):
                    nc.tensor.matmul(
                        ctxp[half][:, col0 - qc : col0 - qc + w],
                        v_sb[:, t, 2 * m + half, :],
                        es[:, half, 0:w],
                        start=(t == 0),
                        stop=(t == tmax),
                    )
                if qi == 0 and t == 1 and pending:
                    pending.pop(0)()
                if (it_base + t) in pop_iters and closures:
                    closures.pop(0)()
            # free the PSUM accumulators promptly (gates the psc ring)
            for h in range(2):
                nc.vector.tensor_copy(raw[:, qi, h, :], ctxp[h])

        # prologue: projections for pair 0
        for ci in range(4):
            qk_chunk(0, ci)
        for tp in range(NT // 2):
            v_pair(0, tp)

        # main pipeline: token-half-major, head pair m inner, 512-query
        # chunks innermost. Closures keep the PE fed during the ACT-paced
        # attention iterations; V projections are split into an early half
        # (key tiles 0-7, needed by the pair's own c=0 pass) and a late
        # half (tiles 8-15, first needed by its c=1 pass).
        for c in range(2):
            for m in range(M):
                if c == 0 and m < M - 1:
                    closures = [
                        (lambda mm=m + 1, cc=ci: qk_chunk(mm, cc))
                        for ci in range(4)
                    ] + [
                        (lambda mm=m + 1, pp=tp: v_pair(mm, pp))
                        for tp in range(4)
                    ]
                elif c == 0:
                    closures = [
                        (lambda pp=tp: v_pair(3, pp)) for tp in range(4, 8)
                    ]
                elif m == 0:
                    closures = [
                        (lambda pp=tp: v_pair(1, pp)) for tp in range(4, 8)
                    ]
                elif m == 1:
                    closures = [
                        (lambda pp=tp: v_pair(2, pp)) for tp in range(4, 8)
                    ] + [(lambda t=j: out_tile(t)) for j in range(3)]
                elif m == 2:
                    closures = [(lambda t=j: out_tile(t)) for j in range(3, 6)]
                else:
                    closures = [(lambda t=j: out_tile(t)) for j in range(6, 8)]
                n_iters = 12 if c == 0 else 28
                n_cl = len(closures)
                pop_iters = {i * n_iters // n_cl for i in range(n_cl)}
                raw = smallp.tile(
                    [65, 2, 2, 512], f32, tag="raw", name=f"raw_{m}_{c}"
                )
                it_base = 0
                for qi in range(2):
                    attn(m, c, qi, raw, closures, pop_iters, it_base)
                    it_base += (1024 * c + 512 * qi) // 128 + 4
                while closures:
                    closures.pop(0)()
                pending.append(lambda r=raw, mm=m, cc=c: norm_finish(r, mm, cc))
        # drain remaining deferred normalizations
        while pending:
            pending.pop(0)()

        # epilogue: remaining output tiles
        for tt in range(8, NT):
            out_tile(tt)

    nc.compile()
    return nc


def _get_program():
    if "nc" not in _CACHE:
        _CACHE["nc"] = _build_program()
    return _CACHE["nc"]


def make_in_maps(x, Wq, Wk, Wv, Wo):
    import ml_dtypes

    bf16 = ml_dtypes.bfloat16
    in_maps = []
    for core in range(NCORES):
        b, hg = core // 2, core % 2
        sl = slice(DPC * hg, DPC * (hg + 1))
        # x: [ci*128+p, k*512+tok] = x[b].T[k*128+p, 512ci+tok]
        xb = np.ascontiguousarray(x[b].T).astype(bf16)
        xr = (
            xb.reshape(KT, 128, 4, 512).transpose(2, 1, 0, 3).reshape(512, 4096)
        )
        # wq/wk/wv: [p, m*1024 + k*128 + d] = W[k*128+p, m*128+d]
        def pack_w(W):
            return np.ascontiguousarray(
                W[:, sl]
                .reshape(KT, 128, M, 128)
                .transpose(1, 2, 0, 3)
                .reshape(128, M * KT * 128)
            ).astype(bf16)

        # wo: [p, m*1024 + o] = Wo[sl][m*128+p, o]
        wo = np.ascontiguousarray(
            Wo[sl, :].reshape(M, 128, D).transpose(1, 0, 2).reshape(128, M * D)
        ).astype(bf16)
        in_maps.append(
            {
                "x": np.ascontiguousarray(xr),
                "wq": pack_w(Wq),
                "wk": pack_w(Wk),
                "wv": pack_w(Wv),
                "wo": wo,
            }
        )
    return in_maps


def kernel(x, Wq, Wk, Wv, Wo, bo):
    global LAST_RESULTS
    from concourse.bass_utils import run_bass_kernel_spmd

    x = np.asarray(x, dtype=np.float32)
    nc = _get_program()
    in_maps = make_in_maps(
        x,
        np.asarray(Wq, np.float32),
        np.asarray(Wk, np.float32),
        np.asarray(Wv, np.float32),
        np.asarray(Wo, np.float32),
    )
    res = run_bass_kernel_spmd(
        nc,
        in_maps,
        list(range(NCORES)),
        trace=bool(int(os.environ.get("KERNEL_TRACE", "0"))),
    )
    LAST_RESULTS = res
    bo = np.asarray(bo, np.float32)
    out = np.empty((B, T, D), np.float32)
    for b in range(B):
        out[b] = (
            res.results[2 * b]["out"].astype(np.float32)
            + res.results[2 * b + 1]["out"].astype(np.float32)
            + bo
        )
    return out


# revision 12
# speedup vs baseline: 1.3054x; 1.0139x over previous
"""Multi-head causal attention (B=4, T=2048, D=1024, H=16, hd=64) on 8 trn2 cores.

Sharding: core = (batch, head_group): 4 batches x 2 head-groups of 8 heads.
Each core computes its batch's attention for its 8 heads plus the partial
output projection; the host sums the two head-group partials per batch and
adds the output bias.

All data bf16 (host-cast); PSUM accumulation fp32. The kernel is a software
pipeline: attention runs query-chunk-major (qc = 512 queries) over head
pairs m, token-half-major overall, with the QK/V projections for the next
head pair (and later the output projection) interleaved into the attention
loop as closures so the tensor engine never idles while the scalar engine
computes the softmax exps (the per-iteration critical resource). The two
S-matmul halves of a head pair run concurrently on disjoint PE row-halves.
Softmax normalization is batched per (token-half, pair): one reciprocal
lane-spread round-trip for all four [65,512] context accumulators, with the
latency-bearing stages deferred into the next attention loop. Layouts:

  xsb  [128, k, tok]        x^T, k = 8 contraction tiles of 128 features
  qt/kt[128, m, tok]        partitions = half*64 + hd for head pair m
  v_sb [128, tt, h, 65]     token-partition V, col 64 = ones (denominator)
  es   [128, 2, 512]        exp(S^T) for both heads of the pair at once
  ctx  [65, 512] PSUM       row 64 = softmax denominator via ones column
"""

import os
import sys

sys.path.insert(0, "/opt/trn_rl_repo")

import numpy as np

B = 4
T = 2048
D = 1024
H = 16
HD = 64
NCORES = 8
HPC = 8          # heads per core
DPC = HPC * HD   # 512
KT = D // 128    # 8 k-tiles
NT = T // 128    # 16 token tiles
M = 4            # head pairs per core

_CACHE = {}
LAST_RESULTS = None


def _build_program():
    from contextlib import ExitStack

    import concourse.bass as bass
    import concourse.tile as tile
    from concourse import bacc, mybir

    f32 = mybir.dt.float32
    bf16 = mybir.dt.bfloat16
    Exp = mybir.ActivationFunctionType.Exp

    nc = bacc.Bacc(
        "TRN2", target_bir_lowering=False, debug=False, num_devices=NCORES
    )
    # host-prepacked layouts (see make_in_maps)
    x_d = nc.dram_tensor("x", [4 * 128, KT * 512], bf16, kind="ExternalInput").ap()
    wq_d = nc.dram_tensor("wq", [128, M * KT * 128], bf16, kind="ExternalInput").ap()
    wk_d = nc.dram_tensor("wk", [128, M * KT * 128], bf16, kind="ExternalInput").ap()
    wv_d = nc.dram_tensor("wv", [128, M * KT * 128], bf16, kind="ExternalInput").ap()
    wo_d = nc.dram_tensor("wo", [128, M * D], bf16, kind="ExternalInput").ap()
    out_d = nc.dram_tensor("out", [T, D], bf16, kind="ExternalOutput").ap()

    with tile.TileContext(nc) as tc, ExitStack() as top:
        persist = top.enter_context(tc.tile_pool(name="persist", bufs=1))
        xsb = persist.tile([128, KT, T], bf16, tag="xsb")
        wqs = persist.tile([128, M, KT, 128], bf16, tag="wqs")
        wks = persist.tile([128, M, KT, 128], bf16, tag="wks")
        wvs = persist.tile([128, M, KT, 128], bf16, tag="wvs")
        wos = persist.tile([128, M, D], bf16, tag="wos")
        qt = persist.tile([128, M, T], bf16, tag="qt")
        kt = persist.tile([128, M, T], bf16, tag="kt")
        v_sb = persist.tile([128, NT, HPC, HD + 1], bf16, tag="v")
        ctx_sb = persist.tile([128, M, T], bf16, tag="ctx")
        tri2 = persist.tile([128, 2, 128], bf16, tag="tri2")

        # ones columns feed the softmax-denominator row of the PV matmul
        nc.vector.memset(v_sb[:, :, :, HD : HD + 1], 1.0)
        # causal keep-mask for the 128-wide diagonal block, one copy per
        # half-plane: tri2[p, :, q] = 1 if q >= p else 0
        nc.vector.memset(tri2, 1.0)
        for i in range(2):
            nc.gpsimd.affine_select(
                out=tri2[:, i, :],
                in_=tri2[:, i, :],
                compare_op=mybir.AluOpType.is_ge,
                fill=0.0,
                base=0,
                pattern=[[1, 128]],
                channel_multiplier=-1,
            )

        # weights on the gpsimd DMA queue, m-major so pair 0 lands first
        wq_r = wq_d.rearrange("p (m k d) -> p m k d", m=M, k=KT)
        wk_r = wk_d.rearrange("p (m k d) -> p m k d", m=M, k=KT)
        wv_r = wv_d.rearrange("p (m k d) -> p m k d", m=M, k=KT)
        for m in range(M):
            for w_sb, w_r in ((wqs, wq_r), (wks, wk_r), (wvs, wv_r)):
                nc.gpsimd.dma_start(out=w_sb[:, m], in_=w_r[:, m])
        nc.gpsimd.dma_start(
            out=wos, in_=wo_d.rearrange("p (m o) -> p m o", m=M)
        )
        # x split across the sync and scalar DMA queues: first chunk per
        # k-tile so the first projection's contraction inputs arrive in
        # consumption order, later chunks in k-halves
        for k in range(KT):
            q = nc.sync if k % 2 == 0 else nc.scalar
            q.dma_start(
                out=xsb[:, k, 0:512], in_=x_d[0:128, 512 * k : 512 * (k + 1)]
            )
        for ci in range(1, 4):
            for kh in range(2):
                q = nc.sync if kh == 0 else nc.scalar
                q.dma_start(
                    out=xsb[:, 4 * kh : 4 * kh + 4, 512 * ci : 512 * (ci + 1)],
                    in_=x_d[
                        128 * ci : 128 * (ci + 1),
                        2048 * kh : 2048 * (kh + 1),
                    ],
                )

        pss = top.enter_context(tc.tile_pool(name="pss", bufs=2, space="PSUM"))
        psc = top.enter_context(tc.tile_pool(name="psc", bufs=4, space="PSUM"))
        expp = top.enter_context(tc.tile_pool(name="expp", bufs=4))
        smallp = top.enter_context(tc.tile_pool(name="smallp", bufs=2))
        outp = top.enter_context(tc.tile_pool(name="outp", bufs=3))

        def qk_chunk(m, ci):
            """Project Q^T and K^T for head pair m over tokens [512ci, 512ci+512)."""
            ps = pss.tile([128, 2, 512], f32, tag="pp", name=f"pqk_{m}_{ci}")
            for j, (w_sb, dest) in enumerate(((wqs, qt), (wks, kt))):
                for k in range(KT):
                    nc.tensor.matmul(
                        ps[:, j, :],
                        w_sb[:, m, k, :],
                        xsb[:, k, 512 * ci : 512 * (ci + 1)],
                        start=(k == 0),
                        stop=(k == KT - 1),
                    )
                nc.vector.tensor_copy(
                    dest[:, m, 512 * ci : 512 * (ci + 1)], ps[:, j, :]
                )

        def v_pair(m, tp):
            """Project V for head pair m over token tiles 2tp, 2tp+1."""
            ps = pss.tile([128, 2, 512], f32, tag="pp", name=f"pv_{m}_{tp}")
            for j in range(2):
                tt = 2 * tp + j
                for k in range(KT):
                    nc.tensor.matmul(
                        ps[:, j, 0:128],
                        xsb[:, k, 128 * tt : 128 * (tt + 1)],
                        wvs[:, m, k, :],
                        start=(k == 0),
                        stop=(k == KT - 1),
                    )
                nc.vector.tensor_copy(
                    v_sb[:, tt, 2 * m : 2 * m + 2, 0:HD],
                    ps[:, j, 0:128].rearrange("p (h c) -> p h c", c=HD),
                )

        def out_tile(tt):
            """Output projection for token tile tt (all four head pairs)."""
            ps = pss.tile([128, 2, 512], f32, tag="pp", name=f"po_{tt}")
            for oc in range(2):
                for ct in range(M):
                    nc.tensor.matmul(
                        ps[:, oc, :],
                        ctx_sb[:, ct, 128 * tt : 128 * (tt + 1)],
                        wos[:, ct, 512 * oc : 512 * (oc + 1)],
                        start=(ct == 0),
                        stop=(ct == M - 1),
                    )
            ot = outp.tile([128, 1024], bf16, tag="ot", name=f"ot_{tt}")
            nc.vector.tensor_copy(
                ot.rearrange("p (a b) -> p a b", a=2), ps
            )
            nc.sync.dma_start(
                out=out_d[128 * tt : 128 * (tt + 1), :], in_=ot
            )

        def norm_finish(raw, m, c, qi=None):
            """Deferred stage of the batched softmax normalization for
            (token half c, pair m): spread the 2048 denominators over 128
            lanes via two small DMA reshapes (a [1,2048] reciprocal on one
            DVE lane would be ~8 cycles/elem), broadcast the reciprocals,
            and scale the context. Broadcast+multiplies live on gpsimd,
            which has no PE-critical work queued. With qi set, process only
            that 512-query half (used to shorten the final drain)."""
            tag = f"{m}_{c}" if qi is None else f"{m}_{c}_{qi}"
            qs = slice(0, 2) if qi is None else slice(qi, qi + 1)
            nq = 2 if qi is None else 1
            den = raw[64:65, qs, :, :]
            sp16 = smallp.tile([128, 16], f32, tag="sp16", name=f"sp_{tag}")
            nc.sync.dma_start(out=sp16[:, 0 : 8 * nq], in_=den)
            rp16 = smallp.tile([128, 16], f32, tag="rp16", name=f"rp_{tag}")
            nc.vector.reciprocal(rp16[:, 0 : 8 * nq], sp16[:, 0 : 8 * nq])
            recip = smallp.tile([1, 2, 2, 512], f32, tag="recip", name=f"rc_{tag}")
            nc.sync.dma_start(out=recip[:, qs, :, :], in_=rp16[:, 0 : 8 * nq])
            bcr = smallp.tile([64, 2, 2, 512], f32, tag="bcr", name=f"bc_{tag}")
            nc.gpsimd.partition_broadcast(bcr[:, qs, :, :], recip[:, qs, :, :])
            for h in range(2):
                nc.gpsimd.tensor_mul(
                    ctx_sb[
                        64 * h : 64 * h + 64,
                        m,
                        1024 * c + 512 * (qi or 0) : 1024 * c + 512 * (qi or 0) + 512 * nq,
                    ].rearrange("p (a b) -> p a b", a=nq),
                    raw[0:64, qs, h, :],
                    bcr[:, qs, h, :],
                )

        pending = []  # deferred norm_finish closures

        def attn(m, c, qi, raw, closures, pop_iters, it_base):
            """Causal attention for head pair m, queries [qc, qc+512)."""
            qc = 1024 * c + 512 * qi
            ctxp = [
                psc.tile([65, 512], f32, tag="ctx", name=f"ctx_{m}_{qc}_{i}")
                for i in range(2)
            ]
            tmax = qc // 128 + 3
            for t in range(tmax + 1):
                j0 = 128 * t
                col0 = max(j0, qc)
                w = qc + 512 - col0
                sp = pss.tile([128, 2, 512], f32, tag="pp", name=f"sp_{m}_{qc}_{t}")
                for half in range(2):
                    pr = 64 * half
                    nc.tensor.matmul(
                        sp[:, half, 0:w],
                        kt[pr : pr + 64, m, j0 : j0 + 128],
                        qt[pr : pr + 64, m, col0 : col0 + w],
                        start=True,
                        stop=True,
                    )
                es = expp.tile(
                    [128, 2, 512], bf16, tag="es", name=f"es_{m}_{qc}_{t}"
                )
                nc.scalar.activation(
                    es[:, :, 0:w], sp[:, :, 0:w], Exp, scale=0.125
                )
                if col0 == j0:
                    # zero the upper triangle of the diagonal block
                    nc.vector.tensor_mul(es[:, :, 0:128], es[:, :, 0:128], tri2)
                for half in range(2):
                    nc.tensor.matmul(
                        ctxp[half][:, col0 - qc : col0 - qc + w],
                        v_sb[:, t, 2 * m + half, :],
                        es[:, half, 0:w],
                        start=(t == 0),
                        stop=(t == tmax),
                    )
                if qi == 0 and t == 1 and pending:
                    pending.pop(0)()
                if (it_base + t) in pop_iters and closures:
                    closures.pop(0)()
            # free the PSUM accumulators promptly (gates the psc ring)
            for h in range(2):
                nc.vector.tensor_copy(raw[:, qi, h, :], ctxp[h])

        # prologue: projections for pair 0
        for ci in range(4):
            qk_chunk(0, ci)
        for tp in range(NT // 2):
            v_pair(0, tp)

        # main pipeline: token-half-major, head pair m inner, 512-query
        # chunks innermost. Closures keep the PE fed during the ACT-paced
        # attention iterations; V projections are split into an early half
        # (key tiles 0-7, needed by the pair's own c=0 pass) and a late
        # half (tiles 8-15, first needed by its c=1 pass).
        for c in range(2):
            for m in range(M):
                if c == 0 and m < M - 1:
                    closures = [
                        (lambda mm=m + 1, cc=ci: qk_chunk(mm, cc))
                        for ci in range(4)
                    ] + [
                        (lambda mm=m + 1, pp=tp: v_pair(mm, pp))
                        for tp in range(4)
                    ]
                elif c == 0:
                    closures = [
                        (lambda pp=tp: v_pair(3, pp)) for tp in range(4, 8)
                    ]
                elif m == 0:
                    closures = [
                        (lambda pp=tp: v_pair(1, pp)) for tp in range(4, 8)
                    ]
                elif m == 1:
                    closures = [
                        (lambda pp=tp: v_pair(2, pp)) for tp in range(4, 8)
                    ] + [(lambda t=j: out_tile(t)) for j in range(3)]
                elif m == 2:
                    closures = [(lambda t=j: out_tile(t)) for j in range(3, 6)]
                else:
                    closures = [(lambda t=j: out_tile(t)) for j in range(6, 8)]
                n_iters = 12 if c == 0 else 28
                n_cl = len(closures)
                pop_iters = {i * n_iters // n_cl for i in range(n_cl)}
                raw = smallp.tile(
                    [65, 2, 2, 512], f32, tag="raw", bufs=3, name=f"raw_{m}_{c}"
                )
                last = c == 1 and m == M - 1
                it_base = 0
                for qi in range(2):
                    attn(m, c, qi, raw, closures, pop_iters, it_base)
                    it_base += (1024 * c + 512 * qi) // 128 + 4
                    if last:
                        # emit per-half so the first half's chain overlaps
                        # the second half's attention, shortening the drain
                        norm_finish(raw, m, c, qi=qi)
                while closures:
                    closures.pop(0)()
                if not last:
                    pending.append(
                        lambda r=raw, mm=m, cc=c: norm_finish(r, mm, cc)
                    )
        # drain remaining deferred normalizations
        while pending:
            pending.pop(0)()

        # epilogue: remaining output tiles (first those gated only on the
        # already-normalized first 512-query half of pair 3)
        for tt in range(8, NT):
            out_tile(tt)

    nc.compile()
    return nc


def _get_program():
    if "nc" not in _CACHE:
        _CACHE["nc"] = _build_program()
    return _CACHE["nc"]


def make_in_maps(x, Wq, Wk, Wv, Wo):
    import ml_dtypes

    bf16 = ml_dtypes.bfloat16
    in_maps = []
    for core in range(NCORES):
        b, hg = core // 2, core % 2
        sl = slice(DPC * hg, DPC * (hg + 1))
        # x: [ci*128+p, k*512+tok] = x[b].T[k*128+p, 512ci+tok]
        xb = np.ascontiguousarray(x[b].T).astype(bf16)
        xr = (
            xb.reshape(KT, 128, 4, 512).transpose(2, 1, 0, 3).reshape(512, 4096)
        )
        # wq/wk/wv: [p, m*1024 + k*128 + d] = W[k*128+p, m*128+d]
        def pack_w(W):
            return np.ascontiguousarray(
                W[:, sl]
                .reshape(KT, 128, M, 128)
                .transpose(1, 2, 0, 3)
                .reshape(128, M * KT * 128)
            ).astype(bf16)

        # wo: [p, m*1024 + o] = Wo[sl][m*128+p, o]
        wo = np.ascontiguousarray(
            Wo[sl, :].reshape(M, 128, D).transpose(1, 0, 2).reshape(128, M * D)
        ).astype(bf16)
        in_maps.append(
            {
                "x": np.ascontiguousarray(xr),
                "wq": pack_w(Wq),
                "wk": pack_w(Wk),
                "wv": pack_w(Wv),
                "wo": wo,
            }
        )
    return in_maps


def kernel(x, Wq, Wk, Wv, Wo, bo):
    global LAST_RESULTS
    from concourse.bass_utils import run_bass_kernel_spmd

    x = np.asarray(x, dtype=np.float32)
    nc = _get_program()
    in_maps = make_in_maps(
        x,
        np.asarray(Wq, np.float32),
        np.asarray(Wk, np.float32),
        np.asarray(Wv, np.float32),
        np.asarray(Wo, np.float32),
    )
    res = run_bass_kernel_spmd(
        nc,
        in_maps,
        list(range(NCORES)),
        trace=bool(int(os.environ.get("KERNEL_TRACE", "0"))),
    )
    LAST_RESULTS = res
    bo = np.asarray(bo, np.float32)
    out = np.empty((B, T, D), np.float32)
    for b in range(B):
        out[b] = (
            res.results[2 * b]["out"].astype(np.float32)
            + res.results[2 * b + 1]["out"].astype(np.float32)
            + bo
        )
    return out
